# revision 115
# baseline (speedup 1.0000x reference)
"""Trainium2 Bass kernel for nn_LlamaAttention_cam (sparse_attention).

Sharding: 16 heads across 8 cores (2 heads/core), both batches per core.
Q/K/V projections column-parallel over heads; o_proj row-parallel (per-core
partial outputs summed on host). The CaM merge is a rank-1 correction
(s_tail outer v_e) applied on host from tiny device-side statistics.

Self-contained: hardcodes all shapes; takes full inputs, returns full output.
"""

import math
import os

import numpy as np
import ml_dtypes

B, T, HID, H = 2, 2048, 2048, 16
D = 128
NCORES = 8
HL = H // NCORES  # heads per core = 2
BT = B * T  # 4096
NF = HID // 128  # 16 f-tiles
SCALE = 1.0 / math.sqrt(D)
RB = int(0.25 * T)  # 512 recent budget
WS = T - RB  # 1536
EVICT = WS - 1  # 1535

# jax.random.uniform(jax.random.key(42), (2,16), float32); bernoulli(key,p) == u < p
U_CONST = np.array(
    [[0.59400654, 0.43801308, 0.6285691, 0.00791204, 0.27834702,
      0.7976179, 0.8521497, 0.9625306, 0.67656493, 0.11104441,
      0.4959929, 0.7311437, 0.18970704, 0.1544199, 0.03802836,
      0.33559263],
     [0.92825687, 0.6123972, 0.49262476, 0.733806, 0.18920851,
      0.15386605, 0.037136197, 0.32930005, 0.9372028, 0.5957513,
      0.4615929, 0.6695677, 0.07019377, 0.39408123, 0.55786455,
      0.35412872]], dtype=np.float32)

BF16 = ml_dtypes.bfloat16

_NC_CACHE = {}


def build_nc():
    import concourse.bacc as bacc
    import concourse.mybir as mybir
    import concourse.tile as tile

    f32 = mybir.dt.float32
    bf16 = mybir.dt.bfloat16
    fp16 = mybir.dt.float16
    EXP = mybir.ActivationFunctionType.Exp

    nc = bacc.Bacc("TRN2", target_bir_lowering=False, debug=False)
    env = os.environ
    SMSPS = env.get("BK_SMSPS", "0") == "1"
    B_QK = int(env.get("BK_QKPS", "2"))
    B_VPS = int(env.get("BK_VPS", "1"))
    B_HSP = int(env.get("BK_HSP", "2"))
    B_SPS = int(env.get("BK_SPS", "2"))
    B_OPS = int(env.get("BK_OPS", "4" if SMSPS else "1"))
    B_ROPE = int(env.get("BK_ROPE", "4"))
    B_PT = int(env.get("BK_PT", "8"))
    B_ACC = int(env.get("BK_ACC", "4"))
    B_OB = int(env.get("BK_OB", "4"))
    OBDVE = env.get("BK_OBDVE", "0") == "1"
    SPLITSP = env.get("BK_SPLITSP", "0") == "1"
    B_PPP = int(env.get("BK_PPP", "2"))  # >0: dedicated oproj psum pool
    SMQK = env.get("BK_SMQK", "0") == "1"
    ROTMODE = int(env.get("BK_ROTMODE", "3"))
    DEFNORM = env.get("BK_DEFNORM", "0") == "1"
    ALLPOOLS = env.get("BK_ALLPOOLS", "1") == "1"
    JIL = env.get("BK_JIL", "0") == "1"  # interleave attn pair j-loops
    TAILGSZ = int(env.get("BK_TAILGSZ", "4"))
    OPSPLIT = env.get("BK_OPSPLIT", "0") == "1"
    WIDEEXP = env.get("BK_WIDEEXP", "0") == "1"
    BCMM = env.get("BK_BCMM", "0") == "1"
    STATG = env.get("BK_STATG", "0") == "1"
    OBMOD = int(env.get("BK_OBMOD", "2"))
    OQK3 = env.get("BK_OQK3", "1") == "1"
    OPP1 = env.get("BK_OPP1", "0") == "1"
    OPP1C = int(env.get("BK_OPP1C", "3"))
    SPP1 = env.get("BK_SPP1", "0") == "1"
    OQK23 = env.get("BK_OQK23", "0") == "1"
    AP0 = env.get("BK_AP0", "1") == "1"
    SM3QK = env.get("BK_SM3QK", "0") == "1"
    AP1C = int(env.get("BK_AP1C", "3"))
    CYCB = int(env.get("BK_CYCB", "2"))
    if WIDEEXP:  # sps holds [128,1024] slots (2 banks each): 2+4+1+1 = 8
        B_PPP = int(env.get("BK_PPP", "1"))
        B_PT = int(env.get("BK_PT", "4"))

    hsT = nc.dram_tensor("hsT", [HID, BT], bf16, kind="ExternalInput")
    wq = nc.dram_tensor("wq", [HID, 256], bf16, kind="ExternalInput")
    wk = nc.dram_tensor("wk", [HID, 256], bf16, kind="ExternalInput")
    wv = nc.dram_tensor("wv", [HID, 256], bf16, kind="ExternalInput")
    wo = nc.dram_tensor("wo", [256, HID], bf16, kind="ExternalInput")
    cosd = nc.dram_tensor("cosT", [128, T], bf16, kind="ExternalInput")
    sind = nc.dram_tensor("sinT", [128, T], bf16, kind="ExternalInput")
    maskd = nc.dram_tensor("masks", [128, 2048], bf16, kind="ExternalInput")

    outT = nc.dram_tensor("outT", [HID, BT], bf16, kind="ExternalOutput")
    abard = nc.dram_tensor("abar", [4, 128, 16], f32, kind="ExternalOutput")
    sumsd = nc.dram_tensor("sums", [4, 2, T], f32, kind="ExternalOutput")

    STATDMA = nc.gpsimd.dma_start if STATG else nc.sync.dma_start
    with tile.TileContext(nc) as tc:
        with (
            tc.tile_pool(name="singles", bufs=1) as singles,
            tc.tile_pool(name="res", bufs=1) as res,
            tc.tile_pool(name="stats", bufs=1) as stats,
            # shared PSUM pools for all phases (8 banks total)
            tc.tile_pool(name="qkps", bufs=B_QK, space="PSUM") as qkps,
            tc.tile_pool(name="vps", bufs=B_VPS, space="PSUM") as vps,
            tc.tile_pool(name="sps", bufs=B_SPS, space="PSUM") as sps,
            tc.tile_pool(name="ops", bufs=B_OPS, space="PSUM") as ops,
            tc.tile_pool(name="ppp", bufs=max(B_PPP, 1), space="PSUM") as ppp,
            # SBUF working pools
            tc.tile_pool(name="hsp", bufs=B_HSP) as hsp,
            tc.tile_pool(name="rope", bufs=B_ROPE) as rope,
            tc.tile_pool(name="pt", bufs=B_PT) as ptp,
            tc.tile_pool(name="wpt", bufs=int(env.get("BK_WPT", "2"))) as wptp,
            tc.tile_pool(name="accp", bufs=B_ACC) as accp,
            tc.tile_pool(name="att_sm", bufs=int(env.get("BK_ATSM", "3"))) as atsm,
            tc.tile_pool(name="ob", bufs=B_OB) as obp,
        ):
            # --- constants: emit DMAs in first-use order ---
            wq_sb = singles.tile([128, NF, 256], bf16, tag="wq")
            wk_sb = singles.tile([128, NF, 256], bf16, tag="wk")
            wv_sb = singles.tile([128, NF, 256], bf16, tag="wv")
            hs_tiles = {}

            def load_hs(c, pieces=2):
                hs_t = hsp.tile([128, NF, 512], bf16, tag="hs")
                np_ = NF // pieces
                for pc in range(pieces):
                    fs = slice(pc * np_, (pc + 1) * np_)
                    nc.sync.dma_start(
                        out=hs_t[:, fs, :],
                        in_=hsT[:, c * 512:(c + 1) * 512]
                        .rearrange("(nf p) t -> p nf t", p=128)[:, fs, :],
                    )
                hs_tiles[c] = hs_t
                return hs_t

            # startup: chunk-0 is processed in half-token passes (q before k)
            # so compute starts after just hs0-half + wq. DMA emission order
            # tracks first use: hs0[t0] + wq, then wk, then hs0[t1].
            hs0 = hsp.tile([128, NF, 512], bf16, tag="hs", name="hs0")
            hs_tiles[0] = hs0

            def _hs0_piece(tpc, fs):
                ts_ = slice(tpc * 256, (tpc + 1) * 256)
                nc.sync.dma_start(
                    out=hs0[:, fs, ts_],
                    in_=hsT[:, 0:512]
                    .rearrange("(nf p) t -> p nf t", p=128)[:, fs, ts_],
                )

            def _w_piece(dst, src, fs):
                nc.sync.dma_start(
                    out=dst[:, fs, :],
                    in_=src.rearrange("(nf p) d -> p nf d", p=128)[:, fs, :],
                )

            # first quarter at f-pair granularity so the very first MM
            # starts after only 0.5MB of DMA; rest at quarters. The wq pieces
            # go out on the idle ACT queue so the HWDGE setups overlap.
            for fp in (slice(0, 2), slice(2, 4)):
                _hs0_piece(0, fp)
                nc.scalar.dma_start(
                    out=wq_sb[:, fp, :],
                    in_=wq.rearrange("(nf p) d -> p nf d", p=128)[:, fp, :],
                )
            for q in range(1, 4):
                fs = slice(q * 4, (q + 1) * 4)
                _hs0_piece(0, fs)
                _w_piece(wq_sb, wq, fs)
            _w_piece(wk_sb, wk, slice(0, 8))
            _w_piece(wk_sb, wk, slice(8, 16))
            _hs0_piece(1, slice(0, 8))
            _hs0_piece(1, slice(8, 16))
            # chunk-0 cos/sin slices: tiny, but they gate RoPE → rope-pool
            # slots → qkps slot release → everything downstream
            cos_sb = singles.tile([128, T], bf16, tag="cos")
            sin_sb = singles.tile([128, T], bf16, tag="sin")
            nc.sync.dma_start(out=cos_sb[:, 0:512], in_=cosd[:, 0:512])
            nc.sync.dma_start(out=sin_sb[:, 0:512], in_=sind[:, 0:512])
            # wv before hs1: chunk-0 V chains fill the gap while hs1 lands
            nc.sync.dma_start(
                out=wv_sb, in_=wv.rearrange("(nf p) d -> p nf d", p=128)
            )
            load_hs(1)
            load_hs(2)
            nc.sync.dma_start(out=cos_sb[:, 512:T], in_=cosd[:, 512:T])
            nc.sync.dma_start(out=sin_sb[:, 512:T], in_=sind[:, 512:T])
            load_hs(3)
            mask_sb = singles.tile([128, 4, 512], bf16, tag="mask")
            nc.sync.dma_start(
                out=mask_sb, in_=maskd.rearrange("p (v t) -> p v t", v=4)
            )
            wo_sb = singles.tile([128, 2, HID], bf16, tag="wo")
            nc.sync.dma_start(
                out=wo_sb, in_=wo.rearrange("(kt p) f -> p kt f", p=128)
            )
            ones_r = singles.tile([1, 128], fp16, tag="onesr")  # bc matmul
            nc.vector.memset(ones_r, 1.0)
            ones_a = singles.tile([128, 2], fp16, tag="onesa")  # [1, 0]
            ones_b = singles.tile([128, 2], fp16, tag="onesb")  # [1, 1]
            nc.vector.memset(ones_a[:, 0:1], 1.0)
            nc.vector.memset(ones_a[:, 1:2], 0.0)
            nc.vector.memset(ones_b, 1.0)

            # --- residents (split per batch for clean phase overlap) ---
            qt = [[res.tile([128, T], bf16, tag=f"qt{b}{h}", name=f"qt{b}{h}")
                   for h in range(HL)] for b in range(B)]
            kt = [[res.tile([128, T], bf16, tag=f"kt{b}{h}", name=f"kt{b}{h}")
                   for h in range(HL)] for b in range(B)]
            vres = [res.tile([128, 16, 256], bf16, tag=f"vres{b}", name=f"vres{b}")
                    for b in range(B)]
            ot = [res.tile([128, T], bf16, tag=f"ot{p}", name=f"ot{p}")
                  for p in range(4)]
            abar_raw = [stats.tile([128, 16], f32, tag=f"ab{p}", name=f"ab{p}")
                        for p in range(4)]

            # ================= phase helpers =================
            def proj_qk(c, t0=0, t1_=512, qk_order=False):
                """Q/K projections + RoPE for tokens [t0,t1_) of chunk c."""
                b, cb = c // 4, c % 4
                hs_t = hs_tiles.get(c) or load_hs(c)
                tw = t1_ - t0
                tl = slice(cb * 512 + t0, cb * 512 + t1_)
                groups = [(wq_sb, qt[b][h]) for h in range(HL)]
                kgroups = [(wk_sb, kt[b][h]) for h in range(HL)]
                if qk_order:  # all q chains before k chains (startup)
                    groups = groups + kgroups
                else:
                    groups = [g for pair in zip(groups, kgroups) for g in pair]
                for w_sb, dest in groups:
                    h = 0 if dest in (qt[b][0], kt[b][0]) else 1
                    ps = qkps.tile([128, 512], f32, tag="qk", name="ps")
                    for f in range(NF):
                        nc.tensor.matmul(
                            ps[:, :tw],
                            lhsT=w_sb[:, f, h * 128:(h + 1) * 128],
                            rhs=hs_t[:, f, t0:t1_],
                            start=(f == 0),
                            stop=(f == NF - 1),
                        )
                    qf = rope.tile([128, 512], bf16, tag="qf", name="qf")
                    nc.scalar.copy(qf[:, :tw], ps[:, :tw])
                    rot = rope.tile([128, 512], bf16, tag="rot", name="rot")
                    if ROTMODE == 1:
                        rdma = [nc.sync.dma_start] * 2
                    elif ROTMODE == 2:
                        rdma = [nc.scalar.dma_start] * 2
                    elif ROTMODE == 3:
                        rdma = [nc.gpsimd.dma_start, nc.scalar.dma_start]
                    elif ROTMODE == 4:  # keep ACT clean during b1 proj
                        rdma = ([nc.gpsimd.dma_start, nc.scalar.dma_start]
                                if b == 0 else
                                [nc.gpsimd.dma_start, nc.sync.dma_start])
                    elif ROTMODE == 5:  # keep HWDGE clean during startup
                        rdma = ([nc.gpsimd.dma_start] * 2 if c <= 1 else
                                [nc.gpsimd.dma_start, nc.scalar.dma_start])
                    elif ROTMODE == 6:  # DVE shuffle network, no DMA at all
                        idm = list(range(32))
                        nc.vector.stream_shuffle(rot[0:64, :tw],
                                                 qf[64:128, :tw], idm)
                        nc.vector.stream_shuffle(rot[64:128, :tw],
                                                 qf[0:64, :tw], idm)
                        rdma = None
                    else:
                        rdma = [nc.gpsimd.dma_start] * 2
                    if rdma is not None:
                        rdma[0](out=rot[0:64, :tw], in_=qf[64:128, :tw])
                        rdma[1](out=rot[64:128, :tw], in_=qf[0:64, :tw])
                    t1 = rope.tile([128, 512], bf16, tag="t1", name="t1")
                    nc.vector.tensor_mul(t1[:, :tw], rot[:, :tw], sin_sb[:, tl])
                    t2 = rope.tile([128, 512], bf16, tag="t2", name="t2")
                    nc.vector.tensor_mul(t2[:, :tw], qf[:, :tw], cos_sb[:, tl])
                    nc.vector.tensor_add(dest[:, tl], t1[:, :tw], t2[:, :tw])

            def proj_v(c):
                """V projection for token chunk c (emitted one chunk late)."""
                b, cb = c // 4, c % 4
                hs_t = hs_tiles[c]
                for s in range(4):
                    vp = qkps.tile([128, 256], f32, tag="qk", name="vp")
                    for f in range(NF):
                        nc.tensor.matmul(
                            vp,
                            lhsT=hs_t[:, f, s * 128:(s + 1) * 128],
                            rhs=wv_sb[:, f, :],
                            start=(f == 0),
                            stop=(f == NF - 1),
                        )
                    nc.scalar.copy(vres[b][:, cb * 4 + s, :], vp)

            def attn_chunk_gen(p, c):
                """Attention for (b,h) pair p, query chunk c (512 queries).

                Generator: yields after each j-step so two pairs' pipelines
                can be emitted interleaved (alternating pool allocations →
                both pairs run concurrently instead of FIFO-serialized).
                """
                b, h = p // 2, p % 2
                jmax = 4 * (c + 1)
                cl = slice(c * 512, (c + 1) * 512)
                qtb, ktb = qt[b][h], kt[b][h]
                if (OPSPLIT and p % 2 == 1) or (OPP1 and p == 1 and c < OPP1C):
                    o_ps = ppp.tile([128, 512], f32, tag="pp", name="o_ps")
                elif (OQK3 and p == 3) or (OQK23 and p == 2):
                    o_ps = qkps.tile([128, 512], f32, tag="qk", name="o_ps")
                else:
                    o_ps = ops.tile([128, 512], f32, tag="o", name="o_ps")
                acc_m = accp.tile([128, 512], fp16, tag="accm")
                acc_t = (accp.tile([128, 512], fp16, tag="acct", name="acct")
                         if c == 3 else None)

                if SPLITSP and p == 3:
                    spool, stag = qkps, "qk"
                elif SPP1 and p == 1:
                    spool, stag = ppp, "pp"
                else:
                    spool, stag = sps, "s"

                def emit_sum(j, ns, pt_src, sp_col):
                    if c == 3:
                        nc.vector.tensor_copy(
                            abar_raw[p][:, j:j + 1], sp_col
                        )
                    acc = acc_t if (c == 3 and j >= 12) else acc_m
                    first = (j == 0) or (c == 3 and j == 12)
                    if first:
                        nc.vector.tensor_copy(acc[:, ns], pt_src)
                    else:
                        nc.vector.tensor_add(acc[:, ns], acc[:, ns], pt_src)

                if WIDEEXP:
                    # non-diagonal key blocks in pairs: one [128,1024] score
                    # tile, two matmuls, ONE exp over both blocks
                    for m in range(2 * c):
                        j0, j1 = 2 * m, 2 * m + 1
                        spw = sps.tile([128, 1024], f32, tag="s", name="spw")
                        for i, j in ((0, j0), (1, j1)):
                            nc.tensor.matmul(
                                spw[:, i * 512:(i + 1) * 512],
                                lhsT=ktb[:, j * 128:(j + 1) * 128],
                                rhs=qtb[:, c * 512:(c + 1) * 512],
                                start=True, stop=True,
                            )
                        ptw = wptp.tile([128, 1024], bf16, tag="pw",
                                        name="ptw")
                        nc.scalar.activation(ptw, spw, EXP, scale=SCALE)
                        for i, j in ((0, j0), (1, j1)):
                            ph = ptw[:, i * 512:(i + 1) * 512]
                            nc.tensor.matmul(
                                o_ps,
                                lhsT=vres[b][:, j, h * 128:(h + 1) * 128],
                                rhs=ph,
                                start=(j == 0), stop=False,
                            )
                            emit_sum(j, slice(0, 512), ph,
                                     spw[:, i * 512 + 511:i * 512 + 512])
                        yield
                    jstart = 4 * c
                else:
                    jstart = 0

                for j in range(jstart, jmax):
                    n0 = 128 * (j - 4 * c) if j >= 4 * c else 0
                    ns = slice(n0, 512)
                    sp = spool.tile([128, 512], f32, tag=stag, name="sp")
                    nc.tensor.matmul(
                        sp[:, ns],
                        lhsT=ktb[:, j * 128:(j + 1) * 128],
                        rhs=qtb[:, c * 512 + n0:(c + 1) * 512],
                        start=True,
                        stop=True,
                    )
                    pt_t = ptp.tile([128, 512], bf16, tag="p")
                    nc.scalar.activation(pt_t[:, ns], sp[:, ns], EXP, scale=SCALE)
                    if j >= 4 * c:
                        nc.vector.tensor_mul(
                            pt_t[:, ns], pt_t[:, ns], mask_sb[:, j - 4 * c, ns]
                        )
                    nc.tensor.matmul(
                        o_ps[:, ns],
                        lhsT=vres[b][:, j, h * 128:(h + 1) * 128],
                        rhs=pt_t[:, ns],
                        start=(j == 0),
                        stop=(j == jmax - 1),
                    )
                    # off the PV critical path: abar stat + row-sum accum
                    emit_sum(j, ns, pt_t[:, ns], sp[:, 511:512])
                    yield

                # per-chunk row sums: row0 = total, row1 = tail (j>=12 keys)
                if WIDEEXP and not SMQK:
                    sm_ps = ppp.tile([2, 512], f32, tag="pp", name="sm_ps")
                elif SMSPS:
                    sm_ps = sps.tile([2, 512], f32, tag="s", name="sm_ps")
                elif SMQK or (SM3QK and p == 3):
                    sm_ps = qkps.tile([2, 512], f32, tag="qk", name="sm_ps")
                else:
                    sm_ps = vps.tile([2, 512], f32, tag="sm", name="sm_ps")
                if c == 3:
                    nc.tensor.matmul(sm_ps, lhsT=ones_b, rhs=acc_t,
                                     start=True, stop=False)
                    nc.tensor.matmul(sm_ps, lhsT=ones_a, rhs=acc_m,
                                     start=False, stop=True)
                else:
                    nc.tensor.matmul(sm_ps, lhsT=ones_a, rhs=acc_m,
                                     start=True, stop=True)

                if DEFNORM:
                    # free the o_ps slot with a single copy; normalize from
                    # SBUF afterwards (off the PSUM critical path)
                    ot_raw = atsm.tile([128, 512], bf16, tag="otraw",
                                       name="ot_raw")
                    nc.vector.tensor_copy(ot_raw, o_ps)
                rec = atsm.tile([1, 512], fp16 if BCMM else f32, tag="rec",
                                name="rec")
                with nc.allow_low_precision(reason="fp16 recip for bcast mm"):
                    nc.vector.reciprocal(rec, sm_ps[0:1, :])
                if BCMM:
                    bc = ppp.tile([128, 512], f32, tag="pp", name="bc")
                    with nc.allow_low_precision(reason="bc is exact recip bcast"):
                        nc.tensor.matmul(bc, lhsT=ones_r, rhs=rec,
                                         start=True, stop=True)
                else:
                    bc = atsm.tile([128, 512], f32, tag="bc", name="bc")
                    nc.gpsimd.partition_broadcast(bc, rec)
                if DEFNORM:
                    nc.vector.tensor_mul(ot[p][:, cl], ot_raw, bc)
                else:
                    nc.vector.tensor_mul(ot[p][:, cl], o_ps, bc)
                sm_sb = atsm.tile([2, 512], f32, tag="smsb")
                nc.vector.tensor_copy(sm_sb, sm_ps)
                STATDMA(out=sumsd[p, :, cl], in_=sm_sb)
                yield

            def attn_chunk(p, c):
                for _ in attn_chunk_gen(p, c):
                    pass

            def attn_pair(p0, p1, c):
                """Emit two pairs' attention j-loops interleaved."""
                gens = [attn_chunk_gen(p0, c), attn_chunk_gen(p1, c)]
                while gens:
                    for g in list(gens):
                        if next(g, "DONE") == "DONE":
                            gens.remove(g)

            def oproj_group(b, c, g, gsz=4, allpools=False):
                tl = slice(c * 512, (c + 1) * 512)
                ob_full = obp.tile([128, 4, 512], bf16, tag="ob", name="ob")
                ob = ob_full[:, :gsz, :]
                for i in range(gsz):
                    fo = g * gsz + i
                    fs = slice(fo * 128, (fo + 1) * 128)
                    if allpools:
                        # tail chunk: every other psum pool is idle by now —
                        # cycle through them for max pp pipelining
                        if CYCB == 4:
                            cyc = [(ppp, "pp"), (ops, "o"), (ppp, "pp"),
                                   (sps, "s"), (sps, "s"), (qkps, "qk")]
                        elif CYCB == 5:
                            cyc = [(ppp, "pp"), (ppp, "pp"), (sps, "s"),
                                   (ops, "o"), (sps, "s"), (qkps, "qk")]
                        elif CYCB == 6:
                            cyc = [(ppp, "pp"), (ppp, "pp"), (ops, "o"),
                                   (sps, "s"), (sps, "s"), (qkps, "qk")]
                        elif CYCB == 7:
                            cyc = [(ppp, "pp"), (ppp, "pp"), (ops, "o"),
                                   (sps, "s"), (vps, "sm"), (sps, "s"),
                                   (qkps, "qk")]
                        elif CYCB == 2:
                            cyc = [(ppp, "pp"), (ppp, "pp"), (ops, "o"),
                                   (sps, "s"), (qkps, "qk"), (sps, "s")]
                        elif CYCB == 3:
                            cyc = [(ppp, "pp"), (ops, "o"), (ppp, "pp"),
                                   (qkps, "qk"), (sps, "s"), (sps, "s")]
                        elif CYCB:
                            cyc = [(ppp, "pp"), (ops, "o"), (ppp, "pp"),
                                   (sps, "s"), (qkps, "qk"), (sps, "s")]
                        else:
                            cyc = [(ppp, "pp"), (sps, "s"), (qkps, "qk"),
                                   (ppp, "pp"), (sps, "s"), (ops, "o")]
                        pool_, tag_ = cyc[fo % len(cyc)]
                        pp = pool_.tile([128, 512], f32, tag=tag_, name="pp")
                    elif OQK23:
                        cyc = [(ppp, "pp"), (ops, "o"), (ppp, "pp")]
                        pool_, tag_ = cyc[fo % 3]
                        pp = pool_.tile([128, 512], f32, tag=tag_, name="pp")
                    elif B_PPP > 0:
                        pp = ppp.tile([128, 512], f32, tag="pp", name="pp")
                    else:
                        pp = ops.tile([128, 512], f32, tag="o", name="pp")
                    nc.tensor.matmul(
                        pp, lhsT=wo_sb[:, 0, fs], rhs=ot[b * 2 + 0][:, tl],
                        start=True, stop=False,
                    )
                    nc.tensor.matmul(
                        pp, lhsT=wo_sb[:, 1, fs], rhs=ot[b * 2 + 1][:, tl],
                        start=False, stop=True,
                    )
                    if OBDVE or i % OBMOD != OBMOD - 1:
                        nc.vector.tensor_copy(ob[:, i, :], pp)
                    else:
                        nc.scalar.copy(ob[:, i, :], pp)
                cg = slice((b * 4 + c) * 512, (b * 4 + c + 1) * 512)
                outv = outT[:, cg].rearrange("(nf p) t -> p nf t", p=128)
                if allpools and gsz == 4:
                    # tail: split the store across two DMA queues so the
                    # HWDGE setups overlap and the last piece lands sooner
                    h = gsz // 2
                    g0 = g * gsz
                    nc.sync.dma_start(out=outv[:, g0:g0 + h, :],
                                       in_=ob[:, 0:h, :])
                    nc.sync.dma_start(out=outv[:, g0 + h:g0 + gsz, :],
                                       in_=ob[:, h:gsz, :])
                else:
                    nc.sync.dma_start(
                        out=outv[:, g * gsz:(g + 1) * gsz, :],
                        in_=ob,
                    )

            def oproj_chunk(b, c, gsz=4, allpools=False):
                for g in range(16 // gsz):  # output groups of gsz f-tiles
                    oproj_group(b, c, g, gsz, allpools)

            def abar_export(b):
                for hl in range(HL):
                    p = b * 2 + hl
                    ab_exp = atsm.tile([128, 16], f32, tag="abe")
                    nc.scalar.activation(ab_exp, abar_raw[p], EXP, scale=SCALE)
                    STATDMA(out=abard[p], in_=ab_exp)

            # ================= main schedule =================
            # proj b1 fills attn-b0 bubbles; oproj b0 (delayed) fills attn-b1
            # bubbles; oproj(b1,c) drips right after its attn chunk.
            # V chains trail QK by one chunk so they never head-block QK
            # chains in qkps pool FIFO order while waiting on wv/hs DMAs.
            proj_qk(0, 0, 256, qk_order=True)
            proj_qk(0, 256, 512, qk_order=True)
            proj_v(0)
            for c in range(1, 4):
                proj_qk(c)
                proj_v(c)
            for c in range(4):
                if JIL:
                    attn_pair(0, 1, c)
                else:
                    attn_chunk(0, c)
                    attn_chunk(1, c)
            abar_export(0)
            for c in range(4):
                proj_qk(4 + c)
                proj_v(4 + c)
            corder = ([3, 2, 1, 0] if env.get("BK_REVC", "0") == "1"
                      else [0, 1, 2, 3])
            for c in corder:
                if JIL:
                    attn_pair(2, 3, c)
                else:
                    attn_chunk(2, c)
                    attn_chunk(3, c)
                if c == 3:
                    abar_export(1)
                oproj_chunk(0, c, allpools=(c == 3 and ALLPOOLS and AP0))
                oproj_chunk(1, c, gsz=TAILGSZ if c == 3 else 4,
                            allpools=(c >= AP1C and ALLPOOLS))

    nc.compile()
    return nc


def _get_nc():
    if "nc" not in _NC_CACHE:
        _NC_CACHE["nc"] = build_nc()
    return _NC_CACHE["nc"]


def _host_inputs(hidden_states, q_w, k_w, v_w, o_w):
    """Per-core input dicts."""
    hsT = np.ascontiguousarray(
        hidden_states.reshape(BT, HID).T).astype(BF16)
    inv = 10000.0 ** (-np.arange(64, dtype=np.float64) / 64.0)
    t = np.arange(T, dtype=np.float64)
    fr = t[None, :] * inv[:, None]  # [64, T]
    cosT = np.cos(np.concatenate([fr, fr], 0)).astype(BF16)
    sinT = np.sin(np.concatenate([fr, fr], 0)).astype(np.float32)
    sinT[:64] *= -1.0  # sign-baked for swap-halves rotate
    sinT = sinT.astype(BF16)
    masks = np.zeros((128, 4, 512), dtype=np.float32)
    kk = np.arange(128)[:, None]
    tt = np.arange(512)[None, :]
    for v in range(4):
        masks[:, v, :] = (tt >= 128 * v + kk).astype(np.float32)
    masks = masks.reshape(128, 2048).astype(BF16)

    in_maps = []
    for core in range(NCORES):
        rs = slice(core * 256, (core + 1) * 256)
        in_maps.append({
            "hsT": hsT,
            "wq": np.ascontiguousarray(q_w[rs, :].T).astype(BF16),
            "wk": np.ascontiguousarray(k_w[rs, :].T).astype(BF16),
            "wv": np.ascontiguousarray(v_w[rs, :].T).astype(BF16),
            "wo": np.ascontiguousarray(o_w[:, rs].T).astype(BF16),
            "cosT": cosT,
            "sinT": sinT,
            "masks": masks,
        })
    return in_maps


def _epilogue(out, results, hidden_states, v_w, o_w):
    """Add the CaM rank-1 correction per (b, h) on host."""
    for core in range(NCORES):
        r = results[core]
        for p in range(4):
            b, hl = p // 2, p % 2
            h = core * HL + hl
            rowsum = r["sums"][p][0]  # [T] unnormalized exp row sums
            tails = r["sums"][p][1]
            a_exp = np.asarray(r["abar"][p], np.float64).T.reshape(2048)
            a_bar = a_exp / max(float(rowsum[T - 1]), 1e-30)
            avg_w = max(float(np.mean(a_bar[WS:])), 1e-6)
            prob = float(np.clip(a_bar[EVICT] / avg_w, 0.0, 1.0))
            prob = float(np.nan_to_num(prob, nan=0.0, posinf=1.0, neginf=0.0))
            m = 1.0 if U_CONST[b, h] < prob else 0.0
            if m == 0.0:
                continue
            # exact v_e from fp32 inputs
            v_row = hidden_states[b, EVICT, :] @ v_w[h * D:(h + 1) * D, :].T
            v_e = v_row * (m / RB)  # [D]
            w_e = o_w[:, h * D:(h + 1) * D] @ v_e  # [HID]
            s_tail = (tails / np.maximum(rowsum, 1e-30)).astype(np.float32)
            out[b] += np.outer(s_tail, w_e).astype(np.float32)
    return out


def kernel(hidden_states, attention_mask, q_w, k_w, v_w, o_w):
    from concourse.bass_utils import run_bass_kernel_spmd

    nc = _get_nc()
    in_maps = _host_inputs(hidden_states, q_w, k_w, v_w, o_w)
    trace = bool(int(os.environ.get("BK_TRACE", "0")))
    res = run_bass_kernel_spmd(
        nc, in_maps, core_ids=list(range(NCORES)), trace=trace,
    )
    if trace and res.exec_time_ns is not None:
        print(f"HW exec time: {res.exec_time_ns} ns")
        _NC_CACHE["last_exec_ns"] = res.exec_time_ns
        _NC_CACHE["last_trace"] = res.instructions_and_trace
    results = res.results

    acc = np.zeros((HID, BT), dtype=np.float32)
    for core in range(NCORES):
        acc += np.asarray(results[core]["outT"], np.float32)
    out = np.ascontiguousarray(acc.T).reshape(B, T, HID)
    out = _epilogue(out, results, hidden_states, v_w, o_w)
    return out.astype(np.float32)
